# revision 16
# baseline (speedup 1.0000x reference)
"""AdaPool2D (2x2/stride-2 softmax-weighted pooling) on 8 Trainium2 NeuronCores.

Data-parallel over batch: 32 images -> 4 per core. Each core computes the
exponential-maximum pool (softmax-weighted sum over each 2x2 window):

    em[b,wo,ho,c] = sum_k p_k * e^{p_k} / sum_k e^{p_k},  p_k the 4 window vals

With mask == 1.0 the reference output is exactly em_pool (the eDSCW branch is
multiplied by zero), so the device kernel only computes em_pool; the general
mask path falls back to a host implementation of the blend.

Device layout (per core): input viewed as [896 w-rows, 14336 (h*c)]. Chunks of
128 consecutive w-rows go on SBUF partitions; h-chunks of 32 (x 64 channels =
2048 floats) along the free dim. ScalarE computes E=exp(T), VectorE P=T*E, and
TensorE contracts the 2x2 windows with a constant [128->64] w-pair selector
matrix (float32r) using two PSUM-accumulating matmuls over the even/odd h
views. ScalarE then forms R = exp(-ln(SE)) (Exp and Ln share one activation
table set) and VectorE writes OUT = SP * R. Two consecutive units share the
128 PSUM partitions so the post-matmul ops run full-width.
"""

import sys

if "/opt/trn_rl_repo" not in sys.path:
    sys.path.insert(0, "/opt/trn_rl_repo")

import numpy as np

B, W, H, C = 32, 224, 224, 64
N_CORES = 8
B_LOC = B // N_CORES          # 4 images per core
ROWS = B_LOC * W              # 896 w-rows per core
ROW_F = H * C                 # 14336 floats per w-row
WCH = 128                     # w-rows per chunk (64 output rows)
N_WCH = ROWS // WCH           # 7
HC = 32                       # h values per unit
N_HCH = H // HC               # 7
FD_IN = HC * C                # 2048
FD_OUT = FD_IN // 2           # 1024
N_HO = HC // 2                # 16 output-h per unit

_CACHE = {}


def _build_nc():
    from contextlib import ExitStack

    import concourse.tile as tile
    from concourse import bacc, mybir

    f32 = mybir.dt.float32
    bf16 = mybir.dt.bfloat16
    AF = mybir.ActivationFunctionType

    nc = bacc.Bacc(trn_type="TRN2", target_bir_lowering=False)
    x = nc.declare_dram_parameter("inputs", [B_LOC, W, H, C], f32, isOutput=False)
    y = nc.declare_dram_parameter("out", [B_LOC, W // 2, H // 2, C], f32, isOutput=True)
    xr = x.ap().rearrange("b w h c -> (b w) (h c)")    # [896, 14336]
    yr = y.ap().rearrange("b w h c -> (b w) (h c)")    # [448, 7168]

    units = [(wc, hc) for wc in range(N_WCH) for hc in range(N_HCH)]
    import os as _os
    _gsz = int(_os.environ.get("GROUP_SIZE", "2"))
    GROUPS = []
    _u = 0
    while _u < N_HCH:
        GROUPS.append((_u, min(_gsz, N_HCH - _u)))
        _u += _gsz

    with tile.TileContext(nc) as tc, ExitStack() as ctx:
        const_pool = ctx.enter_context(tc.tile_pool(name="const", bufs=1))
        in_pool = ctx.enter_context(tc.tile_pool(name="inp", bufs=2))
        e_pool = ctx.enter_context(tc.tile_pool(name="ep", bufs=4))
        p_pool = ctx.enter_context(tc.tile_pool(name="pp", bufs=4))
        r_pool = ctx.enter_context(tc.tile_pool(name="rp", bufs=2))
        out_pool = ctx.enter_context(tc.tile_pool(name="op", bufs=3))
        psum_pool = ctx.enter_context(tc.tile_pool(name="ps", bufs=2, space="PSUM"))

        # Selectors: sel[h][p, m] = 1.0 iff m == 64*h + p//2. Summing w-row
        # pairs into PSUM partition half h; the other half receives zeros
        # (harmless under accumulation).
        sels = []
        for h in range(2):
            sf = const_pool.tile([128, 128], f32, tag=f"self{h}")
            nc.vector.memset(sf[:], 1.0)
            # keep where p - 2m + 128h >= 0
            nc.gpsimd.affine_select(
                out=sf[:], in_=sf[:], compare_op=mybir.AluOpType.is_ge,
                fill=0.0, base=128 * h, pattern=[[-2, 128]], channel_multiplier=1,
            )
            # keep where 1 - p + 2m - 128h >= 0
            nc.gpsimd.affine_select(
                out=sf[:], in_=sf[:], compare_op=mybir.AluOpType.is_ge,
                fill=0.0, base=1 - 128 * h, pattern=[[2, 128]], channel_multiplier=-1,
            )
            sr = const_pool.tile([128, 128], bf16, tag=f"selr{h}")
            nc.vector.tensor_copy(sr[:], sf[:])
            sels.append(sr)

        # Lazily-built per-w-chunk tiles: full-row bf16 input (cast during
        # DMA) and per-group E = exp(T), P = T*E batched ops.
        state = {"wc": None, "tb": None, "groups": {}}

        def get_unit(wc, hc):
            if state["wc"] != wc:
                t = in_pool.tile([128, ROW_F], bf16, tag="tb")
                nc.gpsimd.dma_start(t[:], xr[wc * WCH:(wc + 1) * WCH, :])
                state.update(wc=wc, tb=t, groups={})
            gi = hc // _gsz
            if gi not in state["groups"]:
                u0, ulen = GROUPS[gi]
                fd = ulen * FD_IN
                sl = slice(u0 * FD_IN, u0 * FD_IN + fd)
                te = e_pool.tile([128, fd], bf16, tag="te")
                nc.scalar.activation(te[:], state["tb"][:, sl], AF.Exp)
                tp = p_pool.tile([128, fd], bf16, tag="tp")
                nc.vector.tensor_mul(tp[:], state["tb"][:, sl], te[:])
                state["groups"][gi] = (te, tp)
            u0 = GROUPS[gi][0]
            off = (hc - u0) * FD_IN
            te, tp = state["groups"][gi]
            return te[:, off:off + FD_IN], tp[:, off:off + FD_IN]

        for t0 in range(0, len(units), 2):
            pair = units[t0:t0 + 2]
            npair = len(pair)
            pr = 64 * npair

            se = psum_pool.tile([128, FD_OUT], f32)
            sp = psum_pool.tile([128, FD_OUT], f32)

            for half, (wc, hc) in enumerate(pair):
                te_v, tp_v = get_unit(wc, hc)
                for src, dst in ((te_v, se), (tp_v, sp)):
                    v = src.rearrange("p (ho t c) -> p ho t c", t=2, c=C)
                    for nch in range(FD_OUT // 512):
                        out_ap = dst[:, nch * 512:(nch + 1) * 512]
                        for dj in range(2):
                            rhs = v[:, nch * 8:(nch + 1) * 8, dj, :]
                            nc.tensor.matmul(
                                out_ap, sels[half][:], rhs,
                                start=(half == 0 and dj == 0),
                                stop=(half == npair - 1 and dj == 1),
                                skip_group_check=True,
                            )

            tr = r_pool.tile([128, FD_OUT], f32)
            nc.vector.reciprocal_approx_fast(out=tr[:pr], in_=se[:pr])
            to = out_pool.tile([128, FD_OUT], f32)
            nc.vector.tensor_mul(to[:pr], sp[:pr], tr[:pr])

            for half, (wc, hc) in enumerate(pair):
                nc.sync.dma_start(
                    yr[wc * 64:(wc + 1) * 64, hc * FD_OUT:(hc + 1) * FD_OUT],
                    to[64 * half:64 * half + 64, :],
                )

    nc.compile()
    return nc


def _ensure_ntff_hook():
    """Register the axon NTFF profile hook if the image's antenv lacks it."""
    import types

    try:
        import antenv.axon_hooks  # noqa: F401
    except ImportError:
        import antenv

        mod = types.ModuleType("antenv.axon_hooks")
        mod._HOOK = None

        def set_axon_ntff_profile_hook(h, _m=mod):
            _m._HOOK = h

        def get_axon_ntff_profile_hook(_m=mod):
            return _m._HOOK

        mod.set_axon_ntff_profile_hook = set_axon_ntff_profile_hook
        mod.get_axon_ntff_profile_hook = get_axon_ntff_profile_hook
        sys.modules["antenv.axon_hooks"] = mod
        antenv.axon_hooks = mod

    from antenv.axon_hooks import (
        get_axon_ntff_profile_hook,
        set_axon_ntff_profile_hook,
    )

    if get_axon_ntff_profile_hook() is None:
        from trn_agent_boot.trn_boot import _ntff_profile_via_ctypes

        set_axon_ntff_profile_hook(
            _ntff_profile_via_ctypes("/opt/axon/libaxon_pjrt.so")
        )


def _run_em(inputs, trace=False):
    """Run the distributed em-pool kernel. Returns (out, BassKernelResults)."""
    from concourse.bass_utils import run_bass_kernel_spmd

    if trace:
        _ensure_ntff_hook()

    nc = _CACHE.get("nc")
    if nc is None:
        nc = _build_nc()
        _CACHE["nc"] = nc

    shards = inputs.reshape(N_CORES, B_LOC, W, H, C)
    in_maps = [{"inputs": np.ascontiguousarray(shards[i])} for i in range(N_CORES)]
    res = run_bass_kernel_spmd(
        nc, in_maps, core_ids=list(range(N_CORES)), trace=trace
    )
    out = np.concatenate([res.results[i]["out"] for i in range(N_CORES)], axis=0)
    return out, res


def _pool_numpy(inputs):
    """Host reference of both pools (used only when mask != 1)."""
    x = inputs.astype(np.float64)
    bb, w, h, c = x.shape
    p = x.reshape(bb, w // 2, 2, h // 2, 2, c).transpose(0, 1, 3, 2, 4, 5)
    p = p.reshape(bb, w // 2, h // 2, 4, c)
    ew = np.exp(p - p.max(axis=3, keepdims=True))
    ew /= ew.sum(axis=3, keepdims=True)
    em = (p * ew).sum(axis=3)
    x_avg = p.mean(axis=3, keepdims=True)
    dsc = 2.0 * (x_avg * p) / (x_avg * x_avg + p * p)
    dw = np.exp(dsc - dsc.max(axis=3, keepdims=True))
    dw /= dw.sum(axis=3, keepdims=True)
    dp = (p * dw).sum(axis=3)
    return em, dp


def kernel(inputs, mask):
    inputs = np.ascontiguousarray(np.asarray(inputs, dtype=np.float32))
    m = float(np.asarray(mask).reshape(-1)[0])
    if m == 1.0:
        out, _ = _run_em(inputs)
        return out
    em, dp = _pool_numpy(inputs)
    return (em * m + dp * (1.0 - m)).astype(np.float32)
